# revision 7
# baseline (speedup 1.0000x reference)
"""Trainium2 Bass kernel for nn_Block_343597384085.

Model (per batch b):
  c        = silu(causal_depthwise_conv(x, K=4) + conv_b)
  out_gate = silu(x @ gate_w + gate_b)
  v = ctx = out = c
  for i in 0..3:
      cn      = rmsnorm(ctx) * rms_w[i]
      alphas  = sigmoid(cn @ alpha_w[i] + alpha_b[i])
      betas   = silu(cn @ beta_w[i] + beta_b[i])
      ws      = sqrt(clip(1 - alphas^2, 1e-6))
      fetched = assoc_scan(h_t = a_t h_{t-1} + v_t) over (v*betas*ws, alphas)
      ctx     = ctx + silu(fetched @ ctx_w[i] + ctx_b[i])
      out     = out + fetched
  out = rmsnorm(out * out_gate) * fin_rms_w
  y   = silu(out @ fin_w + fin_b)

Sharding: 8 cores = (batch, seq-half). Each core: 1024 tokens x D=1024,
feature-major SBUF layout [D-block(128 part), tokens(free)].
The scan's cross-half carry moves via a per-iteration pair AllGather (4KB);
each core then computes fetched = h_local + cumprod(alpha)*carry, with
carry masked to 0 on even (first-half) cores so the program is uniform.
All matmuls run in float32r (full PE speed, ~1e-4 relative error).
"""
import numpy as np

import concourse.bass as bass
import concourse.bacc as bacc
import concourse.mybir as mybir
import concourse.tile as tile
from concourse import bass_utils

B, S, D, N, K = 4, 2048, 1024, 4, 4
EPS = 1e-6
P = 128                 # partitions per feature block
NB = D // P             # 8 feature blocks
T = S // 2              # tokens per core
SUB = 512               # matmul moving-dim tile (one fp32 PSUM bank)
NS = T // SUB           # sub-tiles per core
F32 = mybir.dt.float32
F32R = mybir.dt.float32r
OP = mybir.AluOpType
AF = mybir.ActivationFunctionType

_CACHE = {}


def _build():
    nc = bacc.Bacc("TRN2", target_bir_lowering=False, debug=False, num_devices=8)

    # per-core inputs
    xh_d = nc.dram_tensor("xh", [T + K - 1, D], F32, kind="ExternalInput")
    mask_d = nc.dram_tensor("mask", [P, 1], F32, kind="ExternalInput")
    # packed per-partition aux: [P, NB] / [P, N*NB] with col = i*NB + nb
    cwp_d = nc.dram_tensor("cwp", [P, NB * K], F32, kind="ExternalInput")
    cbp_d = nc.dram_tensor("cbp", [P, NB], F32, kind="ExternalInput")
    gbp_d = nc.dram_tensor("gbp", [P, NB], F32, kind="ExternalInput")
    rwp_d = nc.dram_tensor("rwp", [P, N * NB], F32, kind="ExternalInput")
    abp_d = nc.dram_tensor("abp", [P, N * NB], F32, kind="ExternalInput")
    bbp_d = nc.dram_tensor("bbp", [P, N * NB], F32, kind="ExternalInput")
    ctbp_d = nc.dram_tensor("ctbp", [P, N * NB], F32, kind="ExternalInput")
    frwp_d = nc.dram_tensor("frwp", [P, NB], F32, kind="ExternalInput")
    fbp_d = nc.dram_tensor("fbp", [P, NB], F32, kind="ExternalInput")
    gw_d = nc.dram_tensor("gate_w", [D, D], F32, kind="ExternalInput")
    aw_d = nc.dram_tensor("alpha_w", [N, D, D], F32, kind="ExternalInput")
    bw_d = nc.dram_tensor("beta_w", [N, D, D], F32, kind="ExternalInput")
    cw_d = nc.dram_tensor("ctx_w", [N, D, D], F32, kind="ExternalInput")
    fw_d = nc.dram_tensor("fin_w", [D, D], F32, kind="ExternalInput")
    y_d = nc.dram_tensor("y", [T, D], F32, kind="ExternalOutput")

    with tile.TileContext(nc) as tc:
        _emit(nc, tc, locals())
    nc.compile()
    return nc


def _emit(nc, tc, t):
    xh_d = t["xh_d"]; mask_d = t["mask_d"]; cwp_d = t["cwp_d"]
    cbp_d = t["cbp_d"]; gbp_d = t["gbp_d"]; rwp_d = t["rwp_d"]
    abp_d = t["abp_d"]; bbp_d = t["bbp_d"]; ctbp_d = t["ctbp_d"]
    frwp_d = t["frwp_d"]; fbp_d = t["fbp_d"]; gw_d = t["gw_d"]
    aw_d = t["aw_d"]; bw_d = t["bw_d"]; cw_d = t["cw_d"]; fw_d = t["fw_d"]
    y_d = t["y_d"]

    import contextlib
    with contextlib.ExitStack() as est:
        aux = est.enter_context(tc.tile_pool(name="aux", bufs=1))
        state = est.enter_context(tc.tile_pool(name="state", bufs=1))
        wp = est.enter_context(tc.tile_pool(name="wp", bufs=2))     # weight slabs
        tmp = est.enter_context(tc.tile_pool(name="tmp", bufs=4))   # [P,SUB] f32 transients
        mmp = est.enter_context(tc.tile_pool(name="mmp", bufs=4, space="PSUM"))
        ssp = est.enter_context(tc.tile_pool(name="ssp", bufs=2, space="PSUM"))
        bcp = est.enter_context(tc.tile_pool(name="bcp", bufs=2, space="PSUM"))
        dram = est.enter_context(tc.tile_pool(name="dram", bufs=1, space="DRAM"))

        # ---- aux constants ----
        def aux_load(name, dram_t, shape):
            tl = aux.tile(shape, F32, name=name)
            nc.sync.dma_start(tl[:], dram_t[:])
            return tl
        mask = aux_load("mask", mask_d, [P, 1])
        cwp = aux_load("cwp", cwp_d, [P, NB * K])
        cbp = aux_load("cbp", cbp_d, [P, NB])
        gbp = aux_load("gbp", gbp_d, [P, NB])
        rwp = aux_load("rwp", rwp_d, [P, N * NB])
        abp = aux_load("abp", abp_d, [P, N * NB])
        bbp = aux_load("bbp", bbp_d, [P, N * NB])
        ctbp = aux_load("ctbp", ctbp_d, [P, N * NB])
        frwp = aux_load("frwp", frwp_d, [P, NB])
        fbp = aux_load("fbp", fbp_d, [P, NB])
        ones_f = aux.tile([P, 1], F32)
        nc.vector.memset(ones_f[:], 1.0)
        ones_r = aux.tile([P, 1], F32R)
        nc.vector.tensor_copy(ones_r[:], ones_f[:])
        ones1_f = aux.tile([1, P], F32)
        nc.vector.memset(ones1_f[:], 1.0)
        ones1_r = aux.tile([1, P], F32R)
        nc.vector.tensor_copy(ones1_r[:], ones1_f[:])
        eps_t = aux.tile([P, 1], F32)
        nc.vector.memset(eps_t[:], EPS)

        # ---- DRAM scratch ----
        v_s = [dram.tile([P, T], F32, name=f"v_s{nb}") for nb in range(NB)]
        og_s = [dram.tile([P, T], F32, name=f"og_s{nb}") for nb in range(NB)]
        oacc = [dram.tile([P, T], F32, name=f"oacc{nb}") for nb in range(NB)]

        # persistent ctx
        ctxb = [state.tile([P, T], F32, name=f"ctx{nb}") for nb in range(NB)]

        def rms_inv(src, sl, tag_suffix):
            """1/sqrt(mean_d(src^2) + eps) broadcast to [P, SUB]."""
            ssps = ssp.tile([1, SUB], F32, tag="ss", name=f"ss{tag_suffix}")
            for nb in range(NB):
                sq = sqr.tile([P, SUB], F32R, tag="sq", name=f"sq{tag_suffix}_{nb}")
                nc.scalar.activation(sq[:], src[nb][:, sl], AF.Square)
                nc.tensor.matmul(ssps[:], ones_r[:], sq[:],
                                 start=(nb == 0), stop=(nb == NB - 1))
            ssr = sqr.tile([1, SUB], F32R, tag="sq", name=f"ssr{tag_suffix}")
            nc.scalar.copy(ssr[:], ssps[:])
            bc = bcp.tile([P, SUB], F32, tag="bc", name=f"bc{tag_suffix}")
            nc.tensor.matmul(bc[:], ones1_r[:], ssr[:], start=True, stop=True)
            sd = tmp.tile([P, SUB], F32, tag="tmp", name=f"sd{tag_suffix}")
            nc.scalar.activation(sd[:], bc[:], AF.Sqrt, bias=eps_t[:, 0:1],
                                 scale=1.0 / D)
            inv = tmp.tile([P, SUB], F32, tag="tmp", name=f"inv{tag_suffix}")
            nc.vector.reciprocal(inv[:], sd[:])
            return inv

        # ---- phase 0: x load (transposed), conv -> v, gate -> out_gate ----
        with tc.tile_pool(name="p0", bufs=1) as p0, \
             tc.tile_pool(name="p0r", bufs=2) as p0r:
            xT = []
            for nb in range(NB):
                xt = p0.tile([P, T + K - 1], F32R, name=f"xT{nb}")
                nc.sync.dma_start(
                    xt[:],
                    xh_d[:, nb * P:(nb + 1) * P].bitcast(F32R).rearrange("a b -> b a"))
                xT.append(xt)

            for nb in range(NB):
                xf = xT[nb].bitcast(F32)
                cacc = p0r.tile([P, T], F32, tag="cacc", name=f"cacc{nb}")
                nc.vector.tensor_scalar(
                    cacc[:], xf[:, 0:T], cwp[:, nb * K:nb * K + 1], None, OP.mult)
                for k in range(1, K):
                    nc.vector.scalar_tensor_tensor(
                        cacc[:], xf[:, k:k + T], cwp[:, nb * K + k:nb * K + k + 1],
                        cacc[:], OP.mult, OP.add)
                vsb = p0r.tile([P, T], F32, tag="vsb", name=f"vsb{nb}")
                nc.scalar.activation(vsb[:], cacc[:], AF.Silu,
                                     bias=cbp[:, nb:nb + 1])
                nc.sync.dma_start(v_s[nb][:], vsb[:])
                nc.sync.dma_start(oacc[nb][:], vsb[:])
                nc.vector.tensor_copy(ctxb[nb][:], vsb[:])

            gw = []
            for k in range(NB):
                gwk = wp.tile([P, D], F32R, tag=f"w{k}", name=f"gw{k}",
                              bufs=2 if k < 4 else 1)
                nc.sync.dma_start(gwk[:], gw_d[k * P:(k + 1) * P, :].bitcast(F32R))
                gw.append(gwk)
            for m in range(NB):
                for s in range(NS):
                    ps = mmp.tile([P, SUB], F32, tag="mm", name=f"psg{m}_{s}")
                    for k in range(NB):
                        nc.tensor.matmul(
                            ps[:], gw[k][:, m * P:(m + 1) * P],
                            xT[k][:, K - 1 + s * SUB:K - 1 + (s + 1) * SUB],
                            start=(k == 0), stop=(k == NB - 1))
                    ogt = tmp.tile([P, SUB], F32, tag="tmp", name=f"og{m}_{s}")
                    nc.scalar.activation(ogt[:], ps[:], AF.Silu,
                                         bias=gbp[:, m:m + 1])
                    nc.sync.dma_start(og_s[m][:, s * SUB:(s + 1) * SUB], ogt[:])

        # iteration pools (opened after phase-0 scope frees its SBUF)
        cfp = est.enter_context(tc.tile_pool(name="cf", bufs=1))    # cn/fetched/fo (f32r)
        alp = est.enter_context(tc.tile_pool(name="alp", bufs=1))   # alphas (+final reuse)
        sip = est.enter_context(tc.tile_pool(name="sip", bufs=1))   # scan_in/h (+final reuse)
        vwp = est.enter_context(tc.tile_pool(name="vwp", bufs=1))   # v stream
        sqr = est.enter_context(tc.tile_pool(name="sqr", bufs=2))   # [P,SUB] f32r transients

        # ---- iterations ----
        for i in range(N):
            # R: cn = rmsnorm(ctx) * rms_w[i]
            cn = [cfp.tile([P, T], F32R, tag=f"cf{nb}", name=f"cn{i}_{nb}")
                  for nb in range(NB)]
            for s in range(NS):
                sl = slice(s * SUB, (s + 1) * SUB)
                inv = rms_inv(ctxb, sl, f"r{i}_{s}")
                for nb in range(NB):
                    nc.vector.scalar_tensor_tensor(
                        cn[nb][:, sl], ctxb[nb][:, sl],
                        rwp[:, i * NB + nb:i * NB + nb + 1], inv[:],
                        OP.mult, OP.mult)

            # A: alphas = sigmoid(cn @ alpha_w[i] + alpha_b[i])
            wa = []
            for k in range(NB):
                wak = wp.tile([P, D], F32R, tag=f"w{k}", name=f"wa{i}_{k}",
                              bufs=2 if k < 4 else 1)
                nc.sync.dma_start(
                    wak[:], aw_d[i, k * P:(k + 1) * P, :].bitcast(F32R))
                wa.append(wak)
            alphas = [alp.tile([P, T], F32, tag=f"al{nb}", name=f"alphas{i}_{nb}")
                      for nb in range(NB)]
            for m in range(NB):
                for s in range(NS):
                    sl = slice(s * SUB, (s + 1) * SUB)
                    ps = mmp.tile([P, SUB], F32, tag="mm", name=f"psa{i}_{m}_{s}")
                    for k in range(NB):
                        nc.tensor.matmul(ps[:], wa[k][:, m * P:(m + 1) * P],
                                         cn[k][:, sl],
                                         start=(k == 0), stop=(k == NB - 1))
                    nc.scalar.activation(alphas[m][:, sl], ps[:], AF.Sigmoid,
                                         bias=abp[:, i * NB + m:i * NB + m + 1])

            # B: scan_in = v * silu(cn@beta_w+b) * sqrt(1-alphas^2);
            #    then in-place h-scan per block, carry = last column
            wb = []
            for k in range(NB):
                wbk = wp.tile([P, D], F32R, tag=f"w{k}", name=f"wb{i}_{k}",
                              bufs=2 if k < 4 else 1)
                nc.sync.dma_start(
                    wbk[:], bw_d[i, k * P:(k + 1) * P, :].bitcast(F32R))
                wb.append(wbk)
            sin = [sip.tile([P, T], F32, tag=f"sin{nb}", name=f"sin{i}_{nb}")
                   for nb in range(NB)]
            carries = aux.tile([P, NB], F32, name=f"carries{i}")
            for m in range(NB):
                vw = vwp.tile([P, T], F32, tag="vw", name=f"vw{i}_{m}")
                nc.sync.dma_start(vw[:], v_s[m][:])
                for s in range(NS):
                    sl = slice(s * SUB, (s + 1) * SUB)
                    ps = mmp.tile([P, SUB], F32, tag="mm", name=f"psb{i}_{m}_{s}")
                    for k in range(NB):
                        nc.tensor.matmul(ps[:], wb[k][:, m * P:(m + 1) * P],
                                         cn[k][:, sl],
                                         start=(k == 0), stop=(k == NB - 1))
                    bet = tmp.tile([P, SUB], F32, tag="tmp", name=f"bet{i}_{m}_{s}")
                    nc.scalar.activation(bet[:], ps[:], AF.Silu,
                                         bias=bbp[:, i * NB + m:i * NB + m + 1])
                    asq = tmp.tile([P, SUB], F32, tag="tmp", name=f"asq{i}_{m}_{s}")
                    nc.scalar.activation(asq[:], alphas[m][:, sl], AF.Square)
                    # ws = sqrt(1 - alphas^2), in place over asq
                    nc.vector.tensor_scalar(asq[:], asq[:], -1.0, 1.0,
                                            OP.mult, OP.add)
                    nc.scalar.activation(asq[:], asq[:], AF.Sqrt)
                    # scan_in = (betas * ws) * v, in place over bet
                    nc.vector.tensor_tensor(bet[:], bet[:], asq[:], OP.mult)
                    nc.vector.tensor_tensor(sin[m][:, sl], bet[:], vw[:, sl],
                                            OP.mult)
                # local scan (initial 0), in place; carry = last column
                nc.vector.tensor_tensor_scan(sin[m][:], alphas[m][:], sin[m][:],
                                             0.0, OP.mult, OP.add)
                nc.vector.tensor_copy(carries[:, m:m + 1], sin[m][:, T - 1:T])

            # carry exchange: pair AllGather; c_eff = mask * even-partner carry
            cin = dram.tile([D], F32, name=f"cin{i}")
            cout = dram.tile([2, D], F32, name=f"cout{i}")
            nc.sync.dma_start(cin[:].rearrange("(nb p) -> p nb", p=P), carries[:])
            nc.gpsimd.collective_compute(
                "AllGather", OP.bypass,
                replica_groups=[[0, 1], [2, 3], [4, 5], [6, 7]],
                ins=[cin.opt()], outs=[cout.opt()])
            gsb = aux.tile([P, NB], F32, name=f"gsb{i}")
            nc.sync.dma_start(
                gsb[:], cout[0:1, :].rearrange("a (nb p) -> (a p) nb", p=P))
            ceff = aux.tile([P, NB], F32, name=f"ceff{i}")
            nc.vector.tensor_scalar(ceff[:], gsb[:], mask[:, 0:1], None, OP.mult)

            # correction: g = cumprod(alphas)*c (in place over alphas),
            # fetched = h_local + g  (f32r, into the freed cn slots)
            fetched = []
            for nb in range(NB):
                nc.vector.tensor_tensor_scan(
                    alphas[nb][:], alphas[nb][:], alphas[nb][:],
                    ceff[:, nb:nb + 1], OP.mult, OP.bypass)
                fe = cfp.tile([P, T], F32R, tag=f"cf{nb}", name=f"fe{i}_{nb}")
                nc.vector.tensor_tensor(fe[:], sin[nb][:], alphas[nb][:], OP.add)
                fetched.append(fe)
                # out += fetched (DMA accumulate into DRAM)
                nc.gpsimd.dma_start(oacc[nb][:], fe.bitcast(F32)[:],
                                    accum_op=OP.add)

            # C: ctx += silu(fetched @ ctx_w[i] + ctx_b[i])
            wc = []
            for k in range(NB):
                wck = wp.tile([P, D], F32R, tag=f"w{k}", name=f"wc{i}_{k}",
                              bufs=2 if k < 4 else 1)
                nc.sync.dma_start(
                    wck[:], cw_d[i, k * P:(k + 1) * P, :].bitcast(F32R))
                wc.append(wck)
            for m in range(NB):
                for s in range(NS):
                    sl = slice(s * SUB, (s + 1) * SUB)
                    ps = mmp.tile([P, SUB], F32, tag="mm", name=f"psc{i}_{m}_{s}")
                    for k in range(NB):
                        nc.tensor.matmul(ps[:], wc[k][:, m * P:(m + 1) * P],
                                         fetched[k][:, sl],
                                         start=(k == 0), stop=(k == NB - 1))
                    cu = tmp.tile([P, SUB], F32, tag="tmp", name=f"cu{i}_{m}_{s}")
                    nc.scalar.activation(cu[:], ps[:], AF.Silu,
                                         bias=ctbp[:, i * NB + m:i * NB + m + 1])
                    nc.gpsimd.tensor_tensor(ctxb[m][:, sl], ctxb[m][:, sl],
                                            cu[:], OP.add)

        # ---- final: y = silu(rmsnorm(out*gate)*fin_rms_w @ fin_w + fin_b)
        po = [sip.tile([P, T], F32, tag=f"sin{nb}", name=f"po{nb}")
              for nb in range(NB)]
        for nb in range(NB):
            ogl = vwp.tile([P, T], F32, tag="vw", name=f"ogl{nb}")
            nc.sync.dma_start(ogl[:], og_s[nb][:])
            oal = alp.tile([P, T], F32, tag=f"al{nb}", name=f"oal{nb}")
            nc.sync.dma_start(oal[:], oacc[nb][:])
            nc.vector.tensor_tensor(po[nb][:], oal[:], ogl[:], OP.mult)
        fo = [cfp.tile([P, T], F32R, tag=f"cf{nb}", name=f"fo{nb}")
              for nb in range(NB)]
        for s in range(NS):
            sl = slice(s * SUB, (s + 1) * SUB)
            inv = rms_inv(po, sl, f"f{s}")
            for nb in range(NB):
                nc.vector.scalar_tensor_tensor(
                    fo[nb][:, sl], po[nb][:, sl], frwp[:, nb:nb + 1], inv[:],
                    OP.mult, OP.mult)
        fw = []
        for k in range(NB):
            fwk = wp.tile([P, D], F32R, tag=f"w{k}", name=f"fw{k}",
                          bufs=2 if k < 4 else 1)
            nc.sync.dma_start(fwk[:], fw_d[k * P:(k + 1) * P, :].bitcast(F32R))
            fw.append(fwk)
        for m in range(NB):
            for s in range(NS):
                sl = slice(s * SUB, (s + 1) * SUB)
                ps = mmp.tile([P, SUB], F32, tag="mm", name=f"psf{m}_{s}")
                for k in range(NB):
                    nc.tensor.matmul(ps[:], fw[k][:, m * P:(m + 1) * P],
                                     fo[k][:, sl],
                                     start=(k == 0), stop=(k == NB - 1))
                yt = tmp.tile([P, SUB], F32, tag="tmp", name=f"yt{m}_{s}")
                nc.scalar.activation(yt[:], ps[:], AF.Silu, bias=fbp[:, m:m + 1])
                nc.sync.dma_start(
                    y_d[s * SUB:(s + 1) * SUB, m * P:(m + 1) * P]
                    .rearrange("a b -> b a"),
                    yt[:])


def _prep_in_maps(inputs):
    x = np.asarray(inputs["x"], np.float32)
    conv_w = np.asarray(inputs["conv_w"], np.float32)
    conv_b = np.asarray(inputs["conv_b"], np.float32)
    gate_w = np.asarray(inputs["gate_w"], np.float32)
    gate_b = np.asarray(inputs["gate_b"], np.float32)
    rms_w = np.asarray(inputs["rms_w"], np.float32)
    alpha_w = np.asarray(inputs["alpha_w"], np.float32)
    alpha_b = np.asarray(inputs["alpha_b"], np.float32)
    beta_w = np.asarray(inputs["beta_w"], np.float32)
    beta_b = np.asarray(inputs["beta_b"], np.float32)
    ctx_w = np.asarray(inputs["ctx_w"], np.float32)
    ctx_b = np.asarray(inputs["ctx_b"], np.float32)
    fin_rms_w = np.asarray(inputs["fin_rms_w"], np.float32)
    fin_w = np.asarray(inputs["fin_w"], np.float32)
    fin_b = np.asarray(inputs["fin_b"], np.float32)

    def pack1(a):       # [D] -> [P, NB]
        return np.ascontiguousarray(a.reshape(NB, P).T)

    def packN(a):       # [N, D] -> [P, N*NB]
        return np.ascontiguousarray(
            a.reshape(N, NB, P).transpose(2, 0, 1).reshape(P, N * NB))

    cwp = np.ascontiguousarray(
        conv_w.T.reshape(NB, P, K).transpose(1, 0, 2).reshape(P, NB * K))
    shared = dict(
        cwp=cwp, cbp=pack1(conv_b), gbp=pack1(gate_b),
        rwp=packN(rms_w), abp=packN(alpha_b), bbp=packN(beta_b),
        ctbp=packN(ctx_b), frwp=pack1(fin_rms_w), fbp=pack1(fin_b),
        gate_w=np.ascontiguousarray(gate_w),
        alpha_w=np.ascontiguousarray(alpha_w),
        beta_w=np.ascontiguousarray(beta_w),
        ctx_w=np.ascontiguousarray(ctx_w),
        fin_w=np.ascontiguousarray(fin_w),
    )
    in_maps = []
    for c in range(8):
        b, h = c // 2, c % 2
        t0 = h * T
        xh = np.zeros((T + K - 1, D), np.float32)
        lo = max(0, t0 - (K - 1))
        xh[(K - 1) - (t0 - lo):] = x[b, lo:t0 + T]
        m = dict(shared)
        m["xh"] = xh
        m["mask"] = np.full((P, 1), float(h), np.float32)
        in_maps.append(m)
    return in_maps


def kernel(**inputs) -> np.ndarray:
    if "nc" not in _CACHE:
        _CACHE["nc"] = _build()
    nc = _CACHE["nc"]
    in_maps = _prep_in_maps(inputs)
    res = bass_utils.run_bass_kernel_spmd(nc, in_maps, core_ids=list(range(8)))
    y = np.empty((B, S, D), np.float32)
    for c in range(8):
        b, h = c // 2, c % 2
        y[b, h * T:(h + 1) * T] = res.results[c]["y"]
    return y
